# revision 2
# baseline (speedup 1.0000x reference)
"""Max-pooling over sequence spans — Trainium2 Bass kernel, v3.

Problem: context [B=8, S=4096, H=1024] f32; spans_begin/spans_len [B, 100] i32.
Output [B, 100, H] f32: out[b, n] = max over rows context[b, begin:begin+max(len,1)].

Design (per core; pure batch-data-parallel over 8 cores):
  * Host converts context to bf16 (rel err ~4e-3 << 2e-2 budget) and appends
    K_MAX rows of -inf padding; all device compute and gather traffic is bf16.
  * Spans are cut into chunks processed in TWO rounds, each round a
    [<=128 partitions] x [T_r steps] grid: round B holds one chunk per span
    (the span tail, <= T_B rows, sorted desc so active sets are prefixes);
    round A holds the overflow chunks (<= T_A rows each, also sorted desc).
    Two-round packing cuts the step count from 64 to ~T_A+T_B (~38) while
    every gathered row passes the DVE exactly once.
  * Gathers are CONTIGUOUS BLOCK gathers: one SWDGE launch per batch with a
    single index column; descriptor p reads k consecutive rows starting at
    meta[p].  (On HW the indirect-DMA descriptor covers the whole contiguous
    out element, so a k-column batch needs exactly this form — per-element
    multi-column index APs execute differently on HW than in CoreSim.)
  * Batch widths follow k_g <= l0_g + 1 (ramp 1,2,4,8,16...).  A chunk that
    expires inside batch g (l0 < len <= l0+k) sets its block start to
    len - k, re-reading a few already-applied rows — idempotent under max.
    Chunks already finished (len <= l0) point at the -inf pad rows.  This
    keeps every gathered row a valid row of the right chunk on every core,
    so the union-over-cores schedule is safe with zero masking.
  * Per-step DVE tensor_tensor(max) over the batch's active prefix; two
    alternating accumulators per round; steps 0/1 initialize via tensor_copy
    (4x perf mode) instead of memset+TT.
  * Round A chunk maxima bounce through a scratch DRAM tile and are gathered
    back (one single-index gather per chunk rank, -inf row padding) aligned
    with round B's span order, then folded into the round-B accumulator.
  * Output is written in bf16 in sorted-span order; the host casts to f32
    and inverts the sort permutation.

The schedule (step counts, batch shapes, prefix sizes) is data-dependent and
baked into the compiled program; programs are cached by schedule key.  All
per-core index data flows in as a small int32 "meta" tensor.
"""

import sys
import numpy as np

sys.path.insert(0, "/opt/trn_rl_repo")

B, S, H = 8, 4096, 1024
N_SPANS = 100
MAX_LEN = 64
N_CORES = 8
P = 128
K_MAX = 16          # max batch width == ctx -inf pad rows
S_PAD = S + K_MAX   # padded ctx rows

_cache = {}


# ----------------------------------------------------------------- planning

def _round_chunks(eff_row, T_A, T_B):
    """Chunk sizes for one core's round A."""
    A = []
    for e in eff_row:
        e = int(e)
        if e > T_B:
            i = -(-(e - T_B) // T_A)
            r = e - (i - 1) * T_A
            A.extend([T_A] * (i - 1))
            A.append(r - T_B)
    return A


def _n_l(sizes, T):
    sizes = np.asarray(sizes) if len(sizes) else np.zeros(1, np.int64)
    return np.array([(sizes > l).sum() for l in range(T)], dtype=np.int64)


def _batches(n_l, max_k=4, waste_frac=0.10, waste_abs=32, tail_ramp=()):
    """Rectangular batches over a non-increasing n_l profile with the block
    constraint k_g <= l0_g + 1 (so expiring chunks can clamp to their tail
    block).  tail_ramp caps the last batch widths for pipeline drain.
    Returns list of (l0, k, n)."""
    T = int((np.asarray(n_l) > 0).sum())
    tail_cols = min(sum(tail_ramp), max(T - 4, 0))
    tail = []
    budget = tail_cols
    for tk in tail_ramp:
        if budget <= 0:
            break
        tk = min(tk, budget)
        tail.append(tk)
        budget -= tk
    T_main = T - sum(tail)

    out = []
    l = 0
    while l < T_main:
        cap = min(max_k, l + 1, T_main - l)
        n0 = int(n_l[l])
        j = 1
        pad = 0
        while j < cap:
            extra = n0 - int(n_l[l + j])
            if (pad + extra) > waste_frac * n0 * (j + 1) + waste_abs:
                break
            pad += extra
            j += 1
        out.append((l, j, n0))
        l += j
    for tk in tail:
        tk = min(tk, l + 1)
        out.append((l, tk, int(n_l[l])))
        l += tk
    return out


DVE_TT_NS = 363.0            # bf16 TT per [*,1024] col (measured on HW)
ROW_NS = 2048.0 / 22.5 / 16.0


def _flow_cost(nlA, nlB):
    cost = 0.0
    for nl in (nlA, nlB):
        for n in nl:
            if n > 0:
                cost += max(DVE_TT_NS, float(n) * ROW_NS)
    return cost


def _search_rounds(effs):
    """effs [B, N] -> (T_A, T_B) minimizing projected pipeline cost."""
    best = None
    for T_A in range(4, MAX_LEN + 1):
        for T_B in range(1, T_A + 1):
            nlA = np.zeros(T_A, np.int64)
            nlB = np.zeros(T_B, np.int64)
            ok = True
            for c in range(effs.shape[0]):
                A = _round_chunks(effs[c], T_A, T_B)
                if len(A) > P:
                    ok = False
                    break
                nlA = np.maximum(nlA, _n_l(A, T_A))
                nlB = np.maximum(nlB, _n_l(np.minimum(effs[c], T_B), T_B))
            if not ok:
                continue
            est = _flow_cost(nlA, nlB)
            if best is None or est < best[0]:
                best = (est, T_A, T_B)
    assert best is not None
    return best[1], best[2]


def _plan(spans_begin, spans_len):
    """Host-side schedule for all cores.

    Returns (sched, metas, orders):
      sched: hashable compile schedule
      metas: [B] int32 arrays [P, W] of per-batch block starts + combine idx
      orders: [B, N] span order (eff desc) to invert on output
    """
    eff = np.maximum(spans_len, 1).astype(np.int64)          # [B, N]
    begin = np.clip(spans_begin.astype(np.int64), 0, S - 1)
    T_A, T_B = _search_rounds(eff)

    orders = np.argsort(-eff, axis=1, kind="stable")         # [B, N]

    m_A = 0
    per_core = []
    nlA_u = np.zeros(T_A, np.int64)
    nlB_u = np.zeros(T_B, np.int64)
    n_over_u = 0
    C_A_u = 0
    for c in range(eff.shape[0]):
        o = orders[c]
        e_s = eff[c][o]                                       # desc
        b_s = begin[c][o]
        szB = np.minimum(e_s, T_B)
        begB = b_s + e_s - szB
        A_sz, A_beg, A_span = [], [], []
        for rank in range(len(e_s)):
            e = int(e_s[rank])
            if e <= T_B:
                continue
            i = -(-(e - T_B) // T_A)
            r = e - (i - 1) * T_A
            sizes = [T_A] * (i - 1) + [r - T_B]
            off = 0
            for szc in sizes:
                A_sz.append(szc)
                A_beg.append(int(b_s[rank]) + off)
                A_span.append(rank)
                off += szc
            m_A = max(m_A, i)
        A_sz = np.asarray(A_sz, np.int64)
        A_beg = np.asarray(A_beg, np.int64)
        A_span = np.asarray(A_span, np.int64)
        aord = np.argsort(-A_sz, kind="stable")
        A_sz, A_beg, A_span = A_sz[aord], A_beg[aord], A_span[aord]
        C_A = len(A_sz)
        assert C_A <= P
        n_over = int((e_s > T_B).sum())
        per_core.append((szB, begB, A_sz, A_beg, A_span, C_A, n_over))
        nlA_u = np.maximum(nlA_u, _n_l(A_sz, T_A))
        nlB_u = np.maximum(nlB_u, _n_l(szB, T_B))
        n_over_u = max(n_over_u, n_over)
        C_A_u = max(C_A_u, C_A)

    T_A_eff = int((nlA_u > 0).sum())
    T_B_eff = int((nlB_u > 0).sum())
    nlA_u = nlA_u[:T_A_eff] if T_A_eff else nlA_u[:0]
    nlB_u = nlB_u[:T_B_eff]
    m_A = max(m_A, 1)

    batchesA = tuple(_batches(nlA_u)) if T_A_eff else ()
    batchesB = tuple(_batches(nlB_u, tail_ramp=(4, 2)))

    # rank-j combine gather sizes: spans with > j A-chunks (union)
    n_rank = []
    for j in range(m_A):
        nr = 0
        for c in range(eff.shape[0]):
            cnt = np.bincount(per_core[c][4], minlength=N_SPANS) \
                if len(per_core[c][4]) else np.zeros(N_SPANS, np.int64)
            nr = max(nr, int((cnt > j).sum()))
        n_rank.append(nr)

    def _block_starts(sizes, begs, C, batches, pad_val):
        """[P, n_batches] block-start rows; pad with pad_val."""
        cols = np.full((P, len(batches)), pad_val, np.int64)
        for g, (l0, k, n) in enumerate(batches):
            for p2 in range(min(n, C)):
                cl = int(sizes[p2])
                if cl > l0:
                    cols[p2, g] = begs[p2] + min(l0, cl - k)
        return cols

    metas = []
    W = len(batchesA) + len(batchesB) + m_A
    for c in range(eff.shape[0]):
        szB, begB, A_sz, A_beg, A_span, C_A, n_over = per_core[c]
        colsA = _block_starts(A_sz, A_beg, C_A, batchesA, S)
        colsB = _block_starts(szB, begB, N_SPANS, batchesB, S)
        comb = np.full((P, m_A), P, np.int64)   # P = scratch -inf row
        cnt = np.zeros(N_SPANS, np.int64)
        for a_rank in range(C_A):
            sp = A_span[a_rank]
            comb[sp, cnt[sp]] = a_rank
            cnt[sp] += 1
        meta = np.concatenate([colsA, colsB, comb], axis=1)
        metas.append(meta.astype(np.int32))

    sched = dict(
        T_A=T_A_eff, T_B=T_B_eff, m_A=m_A,
        nlA=tuple(int(x) for x in nlA_u),
        nlB=tuple(int(x) for x in nlB_u),
        batchesA=batchesA, batchesB=batchesB,
        C_A=C_A_u, n_over=n_over_u, n_rank=tuple(n_rank), W=W,
    )
    return sched, metas, orders


# ------------------------------------------------------------------ device

def _sched_key(sched, repeat):
    return (sched["T_A"], sched["T_B"], sched["m_A"], sched["nlA"],
            sched["nlB"], sched["batchesA"], sched["batchesB"],
            sched["C_A"], sched["n_over"], sched["n_rank"], sched["W"],
            repeat)


def _build_v2(sched, repeat=1, io_lite=False):
    """io_lite: timing-only variant — ctx is an Internal (device-resident,
    uninitialized) tensor and the output is a single row, so per-call host
    transfer is tiny and wall-clock repeat-deltas are clean.  Gather indices
    still come from meta, so the DMA/compute work is identical."""
    import concourse.bass as bass
    import concourse.bacc as bacc
    import concourse.mybir as mybir
    import concourse.tile as tile

    bf16 = mybir.dt.bfloat16
    T_A, T_B, m_A = sched["T_A"], sched["T_B"], sched["m_A"]
    nlA, nlB = sched["nlA"], sched["nlB"]
    W = sched["W"]
    C_A, n_over = sched["C_A"], sched["n_over"]
    n_rank = sched["n_rank"]
    nbA = len(sched["batchesA"])
    K_M = max([k for _, k, _ in sched["batchesA"] + sched["batchesB"]] or [1])

    nc = bacc.Bacc("TRN2", target_bir_lowering=False, debug=False,
                   num_devices=N_CORES)
    if io_lite:
        ctx_d = nc.dram_tensor("ctxi", [S_PAD, H], bf16)
        out_d = nc.dram_tensor("out", [1, H], bf16, kind="ExternalOutput")
    else:
        ctx_d = nc.dram_tensor("ctx", [S_PAD, H], bf16, kind="ExternalInput")
        out_d = nc.dram_tensor("out", [N_SPANS, H], bf16,
                               kind="ExternalOutput")
    meta_d = nc.dram_tensor("meta", [P, W], mybir.dt.int32,
                            kind="ExternalInput")

    MaxOp = mybir.AluOpType.max

    with tile.TileContext(nc) as tc:
        with (
            tc.tile_pool(name="persist", bufs=1) as persist,
            tc.tile_pool(name="slabs", bufs=6) as slabs,
            tc.tile_pool(name="dram", bufs=1, space="DRAM") as dpool,
        ):
            scratch = dpool.tile([P + 1, H], bf16, tag="scratch")
            meta_t = persist.tile([P, W], mybir.dt.int32, tag="meta")
            nc.sync.dma_start(out=meta_t[:], in_=meta_d[:])
            # fill scratch rows [C_A : P+1] with -inf once (combine padding
            # + keeps the full-tensor gather view initialized)
            ninf = persist.tile([P, H], bf16, tag="ninf")
            nc.gpsimd.memset(ninf[:], -3.0e38)
            n_pad = P + 1 - C_A
            nc.sync.dma_start(out=scratch[C_A:P + 1, :], in_=ninf[0:n_pad, :])

            def run_round(gcol0, nl, batches, tag):
                """Emit one round's block gathers + DVE chain."""
                accs = [None, None]
                for g, (l0, k, n) in enumerate(batches):
                    slab = slabs.tile([P, K_M * H], bf16, tag="slab")
                    nc.gpsimd.indirect_dma_start(
                        out=slab[0:n, 0:k * H],
                        out_offset=None,
                        in_=ctx_d[:],
                        in_offset=bass.IndirectOffsetOnAxis(
                            ap=meta_t[0:n, gcol0 + g:gcol0 + g + 1], axis=0),
                    )
                    for j in range(k):
                        l = l0 + j
                        # bound by the BATCH's row count, not nl[l]: a chunk
                        # expiring mid-batch carries its tail rows in later
                        # columns (block clamp) and must stay included;
                        # finished rows read -inf / re-read valid rows.
                        col = slab[0:n, j * H:(j + 1) * H]
                        if l <= 1:
                            acc = persist.tile([P, H], bf16,
                                               tag=f"acc{tag}{l}")
                            nc.vector.tensor_copy(out=acc[0:n, :], in_=col)
                            accs[l] = acc
                        else:
                            a = accs[l % 2]
                            nc.vector.tensor_tensor(
                                out=a[0:n, :], in0=a[0:n, :], in1=col, op=MaxOp)
                return accs

            for _ in range(repeat):
                have_A = T_A > 0 and C_A > 0
                if have_A:
                    accA0, accA1 = run_round(0, nlA, sched["batchesA"], "A")
                    if T_A > 1:
                        m = nlA[1]
                        nc.vector.tensor_tensor(
                            out=accA0[0:m, :], in0=accA0[0:m, :],
                            in1=accA1[0:m, :], op=MaxOp)
                    nc.sync.dma_start(out=scratch[0:C_A, :],
                                      in_=accA0[0:C_A, :])
                    gts = []
                    for j in range(m_A):
                        gt = persist.tile([P, H], bf16, tag=f"gt{j}")
                        nc.gpsimd.indirect_dma_start(
                            out=gt[0:n_rank[j], :],
                            out_offset=None,
                            in_=scratch[:],
                            in_offset=bass.IndirectOffsetOnAxis(
                                ap=meta_t[0:n_rank[j],
                                          nbA + len(sched["batchesB"]) + j:
                                          nbA + len(sched["batchesB"]) + j + 1],
                                axis=0),
                        )
                        gts.append(gt)
                accB0, accB1 = run_round(nbA, nlB, sched["batchesB"], "B")
                if have_A:
                    for j in range(m_A):
                        nc.vector.tensor_tensor(
                            out=accB0[0:n_rank[j], :],
                            in0=accB0[0:n_rank[j], :],
                            in1=gts[j][0:n_rank[j], :], op=MaxOp)
                if T_B > 1:
                    m = nlB[1]
                    nc.vector.tensor_tensor(
                        out=accB0[0:m, :], in0=accB0[0:m, :],
                        in1=accB1[0:m, :], op=MaxOp)
                if io_lite:
                    nc.sync.dma_start(out=out_d[:], in_=accB0[0:1, :])
                else:
                    nc.sync.dma_start(out=out_d[:], in_=accB0[0:N_SPANS, :])
    nc.compile()
    return nc


def _get_v2(sched, repeat=1):
    key = ("v3",) + _sched_key(sched, repeat)
    if key not in _cache:
        _cache[key] = _build_v2(sched, repeat)
    return _cache[key]


# ------------------------------------------------------------------- entry

def _to_bf16_padded(context):
    import ml_dtypes
    ctx16 = np.empty((B, S_PAD, H), dtype=ml_dtypes.bfloat16)
    ctx16[:, :S] = np.asarray(context, dtype=ml_dtypes.bfloat16)
    ctx16[:, S:] = ml_dtypes.bfloat16(-3.0e38)
    return ctx16


def kernel(context, spans_begin, spans_len):
    from concourse.bass_utils import run_bass_kernel_spmd

    spans_begin = np.asarray(spans_begin, dtype=np.int32)
    spans_len = np.asarray(spans_len, dtype=np.int32)
    assert spans_begin.shape == (B, N_SPANS)

    ctx16 = _to_bf16_padded(context)
    sched, metas, orders = _plan(spans_begin, spans_len)
    nc = _get_v2(sched)
    in_maps = [{"ctx": ctx16[b], "meta": metas[b]} for b in range(B)]
    res = run_bass_kernel_spmd(nc, in_maps, list(range(N_CORES)))
    out = np.empty((B, N_SPANS, H), dtype=np.float32)
    for b in range(B):
        out[b, orders[b]] = res.results[b]["out"].astype(np.float32)
    return out


# revision 3
# speedup vs baseline: 1.0445x; 1.0445x over previous
"""Max-pooling over sequence spans — Trainium2 Bass kernel, v3.

Problem: context [B=8, S=4096, H=1024] f32; spans_begin/spans_len [B, 100] i32.
Output [B, 100, H] f32: out[b, n] = max over rows context[b, begin:begin+max(len,1)].

Design (per core; pure batch-data-parallel over 8 cores):
  * Host converts context to bf16 (rel err ~4e-3 << 2e-2 budget) and appends
    K_MAX rows of -inf padding; all device compute and gather traffic is bf16.
  * Spans are cut into chunks processed in TWO rounds, each round a
    [<=128 partitions] x [T_r steps] grid: round B holds one chunk per span
    (the span tail, <= T_B rows, sorted desc so active sets are prefixes);
    round A holds the overflow chunks (<= T_A rows each, also sorted desc).
    Two-round packing cuts the step count from 64 to ~T_A+T_B (~38) while
    every gathered row passes the DVE exactly once.
  * Gathers are CONTIGUOUS BLOCK gathers: one SWDGE launch per batch with a
    single index column; descriptor p reads k consecutive rows starting at
    meta[p].  (On HW the indirect-DMA descriptor covers the whole contiguous
    out element, so a k-column batch needs exactly this form — per-element
    multi-column index APs execute differently on HW than in CoreSim.)
  * Batch widths follow k_g <= l0_g + 1 (ramp 1,2,4,8,16...).  A chunk that
    expires inside batch g (l0 < len <= l0+k) sets its block start to
    len - k, re-reading a few already-applied rows — idempotent under max.
    Chunks already finished (len <= l0) point at the -inf pad rows.  This
    keeps every gathered row a valid row of the right chunk on every core,
    so the union-over-cores schedule is safe with zero masking.
  * Per-step DVE tensor_tensor(max) over the batch's active prefix; two
    alternating accumulators per round; steps 0/1 initialize via tensor_copy
    (4x perf mode) instead of memset+TT.
  * Round A chunk maxima bounce through a scratch DRAM tile and are gathered
    back (one single-index gather per chunk rank, -inf row padding) aligned
    with round B's span order, then folded into the round-B accumulator.
  * Output is written in bf16 in sorted-span order; the host casts to f32
    and inverts the sort permutation.

The schedule (step counts, batch shapes, prefix sizes) is data-dependent and
baked into the compiled program; programs are cached by schedule key.  All
per-core index data flows in as a small int32 "meta" tensor.
"""

import sys
import numpy as np

sys.path.insert(0, "/opt/trn_rl_repo")

B, S, H = 8, 4096, 1024
N_SPANS = 100
MAX_LEN = 64
N_CORES = 8
P = 128
K_MAX = 16          # max batch width == ctx -inf pad rows
S_PAD = S + K_MAX   # padded ctx rows

_cache = {}


# ----------------------------------------------------------------- planning

def _round_chunks(eff_row, T_A, T_B):
    """Chunk sizes for one core's round A."""
    A = []
    for e in eff_row:
        e = int(e)
        if e > T_B:
            i = -(-(e - T_B) // T_A)
            r = e - (i - 1) * T_A
            A.extend([T_A] * (i - 1))
            A.append(r - T_B)
    return A


def _n_l(sizes, T):
    sizes = np.asarray(sizes) if len(sizes) else np.zeros(1, np.int64)
    return np.array([(sizes > l).sum() for l in range(T)], dtype=np.int64)


def _batches(n_l, max_k=K_MAX, waste_frac=0.10, waste_abs=32, tail_ramp=()):
    """Rectangular batches over a non-increasing n_l profile with the block
    constraint k_g <= l0_g + 1 (so expiring chunks can clamp to their tail
    block).  tail_ramp caps the last batch widths for pipeline drain.
    Returns list of (l0, k, n)."""
    T = int((np.asarray(n_l) > 0).sum())
    tail_cols = min(sum(tail_ramp), max(T - 4, 0))
    tail = []
    budget = tail_cols
    for tk in tail_ramp:
        if budget <= 0:
            break
        tk = min(tk, budget)
        tail.append(tk)
        budget -= tk
    T_main = T - sum(tail)

    out = []
    l = 0
    while l < T_main:
        cap = min(max_k, l + 1, T_main - l)
        n0 = int(n_l[l])
        j = 1
        pad = 0
        while j < cap:
            extra = n0 - int(n_l[l + j])
            if (pad + extra) > waste_frac * n0 * (j + 1) + waste_abs:
                break
            pad += extra
            j += 1
        out.append((l, j, n0))
        l += j
    for tk in tail:
        tk = min(tk, l + 1)
        out.append((l, tk, int(n_l[l])))
        l += tk
    return out


DVE_TT_NS = 363.0            # bf16 TT per [*,1024] col (measured on HW)
ROW_NS = 2048.0 / 22.5 / 16.0


def _flow_cost(nlA, nlB):
    cost = 0.0
    for nl in (nlA, nlB):
        for n in nl:
            if n > 0:
                cost += max(DVE_TT_NS, float(n) * ROW_NS)
    return cost


def _search_rounds(effs):
    """effs [B, N] -> (T_A, T_B) minimizing projected pipeline cost."""
    best = None
    for T_A in range(4, MAX_LEN + 1):
        for T_B in range(1, T_A + 1):
            nlA = np.zeros(T_A, np.int64)
            nlB = np.zeros(T_B, np.int64)
            ok = True
            for c in range(effs.shape[0]):
                A = _round_chunks(effs[c], T_A, T_B)
                if len(A) > P:
                    ok = False
                    break
                nlA = np.maximum(nlA, _n_l(A, T_A))
                nlB = np.maximum(nlB, _n_l(np.minimum(effs[c], T_B), T_B))
            if not ok:
                continue
            est = _flow_cost(nlA, nlB)
            if best is None or est < best[0]:
                best = (est, T_A, T_B)
    assert best is not None
    return best[1], best[2]


def _plan(spans_begin, spans_len):
    """Host-side schedule for all cores.

    Returns (sched, metas, orders):
      sched: hashable compile schedule
      metas: [B] int32 arrays [P, W] of per-batch block starts + combine idx
      orders: [B, N] span order (eff desc) to invert on output
    """
    eff = np.maximum(spans_len, 1).astype(np.int64)          # [B, N]
    begin = np.clip(spans_begin.astype(np.int64), 0, S - 1)
    T_A, T_B = _search_rounds(eff)

    orders = np.argsort(-eff, axis=1, kind="stable")         # [B, N]

    m_A = 0
    per_core = []
    nlA_u = np.zeros(T_A, np.int64)
    nlB_u = np.zeros(T_B, np.int64)
    n_over_u = 0
    C_A_u = 0
    for c in range(eff.shape[0]):
        o = orders[c]
        e_s = eff[c][o]                                       # desc
        b_s = begin[c][o]
        szB = np.minimum(e_s, T_B)
        begB = b_s + e_s - szB
        A_sz, A_beg, A_span = [], [], []
        for rank in range(len(e_s)):
            e = int(e_s[rank])
            if e <= T_B:
                continue
            i = -(-(e - T_B) // T_A)
            r = e - (i - 1) * T_A
            sizes = [T_A] * (i - 1) + [r - T_B]
            off = 0
            for szc in sizes:
                A_sz.append(szc)
                A_beg.append(int(b_s[rank]) + off)
                A_span.append(rank)
                off += szc
            m_A = max(m_A, i)
        A_sz = np.asarray(A_sz, np.int64)
        A_beg = np.asarray(A_beg, np.int64)
        A_span = np.asarray(A_span, np.int64)
        aord = np.argsort(-A_sz, kind="stable")
        A_sz, A_beg, A_span = A_sz[aord], A_beg[aord], A_span[aord]
        C_A = len(A_sz)
        assert C_A <= P
        n_over = int((e_s > T_B).sum())
        per_core.append((szB, begB, A_sz, A_beg, A_span, C_A, n_over))
        nlA_u = np.maximum(nlA_u, _n_l(A_sz, T_A))
        nlB_u = np.maximum(nlB_u, _n_l(szB, T_B))
        n_over_u = max(n_over_u, n_over)
        C_A_u = max(C_A_u, C_A)

    T_A_eff = int((nlA_u > 0).sum())
    T_B_eff = int((nlB_u > 0).sum())
    nlA_u = nlA_u[:T_A_eff] if T_A_eff else nlA_u[:0]
    nlB_u = nlB_u[:T_B_eff]
    m_A = max(m_A, 1)

    batchesA = tuple(_batches(nlA_u)) if T_A_eff else ()
    batchesB = tuple(_batches(nlB_u, tail_ramp=(4, 2)))

    # rank-j combine gather sizes: spans with > j A-chunks (union)
    n_rank = []
    for j in range(m_A):
        nr = 0
        for c in range(eff.shape[0]):
            cnt = np.bincount(per_core[c][4], minlength=N_SPANS) \
                if len(per_core[c][4]) else np.zeros(N_SPANS, np.int64)
            nr = max(nr, int((cnt > j).sum()))
        n_rank.append(nr)

    def _block_starts(sizes, begs, C, batches, pad_val):
        """[P, n_batches] block-start rows; pad with pad_val."""
        cols = np.full((P, len(batches)), pad_val, np.int64)
        for g, (l0, k, n) in enumerate(batches):
            for p2 in range(min(n, C)):
                cl = int(sizes[p2])
                if cl > l0:
                    cols[p2, g] = begs[p2] + min(l0, cl - k)
        return cols

    metas = []
    W = len(batchesA) + len(batchesB) + m_A
    for c in range(eff.shape[0]):
        szB, begB, A_sz, A_beg, A_span, C_A, n_over = per_core[c]
        colsA = _block_starts(A_sz, A_beg, C_A, batchesA, S)
        colsB = _block_starts(szB, begB, N_SPANS, batchesB, S)
        comb = np.full((P, m_A), P, np.int64)   # P = scratch -inf row
        cnt = np.zeros(N_SPANS, np.int64)
        for a_rank in range(C_A):
            sp = A_span[a_rank]
            comb[sp, cnt[sp]] = a_rank
            cnt[sp] += 1
        meta = np.concatenate([colsA, colsB, comb], axis=1)
        metas.append(meta.astype(np.int32))

    sched = dict(
        T_A=T_A_eff, T_B=T_B_eff, m_A=m_A,
        nlA=tuple(int(x) for x in nlA_u),
        nlB=tuple(int(x) for x in nlB_u),
        batchesA=batchesA, batchesB=batchesB,
        C_A=C_A_u, n_over=n_over_u, n_rank=tuple(n_rank), W=W,
    )
    return sched, metas, orders


# ------------------------------------------------------------------ device

def _sched_key(sched, repeat):
    return (sched["T_A"], sched["T_B"], sched["m_A"], sched["nlA"],
            sched["nlB"], sched["batchesA"], sched["batchesB"],
            sched["C_A"], sched["n_over"], sched["n_rank"], sched["W"],
            repeat)


def _build_v2(sched, repeat=1, io_lite=False):
    """io_lite: timing-only variant — ctx is an Internal (device-resident,
    uninitialized) tensor and the output is a single row, so per-call host
    transfer is tiny and wall-clock repeat-deltas are clean.  Gather indices
    still come from meta, so the DMA/compute work is identical."""
    import concourse.bass as bass
    import concourse.bacc as bacc
    import concourse.mybir as mybir
    import concourse.tile as tile

    bf16 = mybir.dt.bfloat16
    T_A, T_B, m_A = sched["T_A"], sched["T_B"], sched["m_A"]
    nlA, nlB = sched["nlA"], sched["nlB"]
    W = sched["W"]
    C_A, n_over = sched["C_A"], sched["n_over"]
    n_rank = sched["n_rank"]
    nbA = len(sched["batchesA"])
    K_M = max([k for _, k, _ in sched["batchesA"] + sched["batchesB"]] or [1])

    nc = bacc.Bacc("TRN2", target_bir_lowering=False, debug=False,
                   num_devices=N_CORES)
    if io_lite:
        ctx_d = nc.dram_tensor("ctxi", [S_PAD, H], bf16)
        out_d = nc.dram_tensor("out", [1, H], bf16, kind="ExternalOutput")
    else:
        ctx_d = nc.dram_tensor("ctx", [S_PAD, H], bf16, kind="ExternalInput")
        out_d = nc.dram_tensor("out", [N_SPANS, H], bf16,
                               kind="ExternalOutput")
    meta_d = nc.dram_tensor("meta", [P, W], mybir.dt.int32,
                            kind="ExternalInput")

    MaxOp = mybir.AluOpType.max

    with tile.TileContext(nc) as tc:
        with (
            tc.tile_pool(name="persist", bufs=1) as persist,
            tc.tile_pool(name="slabs", bufs=4) as slabs,
            tc.tile_pool(name="dram", bufs=1, space="DRAM") as dpool,
        ):
            scratch = dpool.tile([P + 1, H], bf16, tag="scratch")
            meta_t = persist.tile([P, W], mybir.dt.int32, tag="meta")
            nc.sync.dma_start(out=meta_t[:], in_=meta_d[:])
            # fill scratch rows [C_A : P+1] with -inf once (combine padding
            # + keeps the full-tensor gather view initialized)
            ninf = persist.tile([P, H], bf16, tag="ninf")
            nc.gpsimd.memset(ninf[:], -3.0e38)
            n_pad = P + 1 - C_A
            nc.sync.dma_start(out=scratch[C_A:P + 1, :], in_=ninf[0:n_pad, :])

            def run_round(gcol0, nl, batches, tag):
                """Emit one round's block gathers + DVE chain."""
                accs = [None, None]
                for g, (l0, k, n) in enumerate(batches):
                    slab = slabs.tile([P, K_M * H], bf16, tag="slab")
                    nc.gpsimd.indirect_dma_start(
                        out=slab[0:n, 0:k * H],
                        out_offset=None,
                        in_=ctx_d[:],
                        in_offset=bass.IndirectOffsetOnAxis(
                            ap=meta_t[0:n, gcol0 + g:gcol0 + g + 1], axis=0),
                    )
                    for j in range(k):
                        l = l0 + j
                        # bound by the BATCH's row count, not nl[l]: a chunk
                        # expiring mid-batch carries its tail rows in later
                        # columns (block clamp) and must stay included;
                        # finished rows read -inf / re-read valid rows.
                        col = slab[0:n, j * H:(j + 1) * H]
                        if l <= 1:
                            acc = persist.tile([P, H], bf16,
                                               tag=f"acc{tag}{l}")
                            nc.vector.tensor_copy(out=acc[0:n, :], in_=col)
                            accs[l] = acc
                        else:
                            a = accs[l % 2]
                            nc.vector.tensor_tensor(
                                out=a[0:n, :], in0=a[0:n, :], in1=col, op=MaxOp)
                return accs

            for _ in range(repeat):
                have_A = T_A > 0 and C_A > 0
                if have_A:
                    accA0, accA1 = run_round(0, nlA, sched["batchesA"], "A")
                    if T_A > 1:
                        m = nlA[1]
                        nc.vector.tensor_tensor(
                            out=accA0[0:m, :], in0=accA0[0:m, :],
                            in1=accA1[0:m, :], op=MaxOp)
                    nc.sync.dma_start(out=scratch[0:C_A, :],
                                      in_=accA0[0:C_A, :])
                    gts = []
                    for j in range(m_A):
                        gt = persist.tile([P, H], bf16, tag=f"gt{j}")
                        nc.gpsimd.indirect_dma_start(
                            out=gt[0:n_rank[j], :],
                            out_offset=None,
                            in_=scratch[:],
                            in_offset=bass.IndirectOffsetOnAxis(
                                ap=meta_t[0:n_rank[j],
                                          nbA + len(sched["batchesB"]) + j:
                                          nbA + len(sched["batchesB"]) + j + 1],
                                axis=0),
                        )
                        gts.append(gt)
                accB0, accB1 = run_round(nbA, nlB, sched["batchesB"], "B")
                if have_A:
                    for j in range(m_A):
                        nc.vector.tensor_tensor(
                            out=accB0[0:n_rank[j], :],
                            in0=accB0[0:n_rank[j], :],
                            in1=gts[j][0:n_rank[j], :], op=MaxOp)
                if T_B > 1:
                    m = nlB[1]
                    nc.vector.tensor_tensor(
                        out=accB0[0:m, :], in0=accB0[0:m, :],
                        in1=accB1[0:m, :], op=MaxOp)
                if io_lite:
                    nc.sync.dma_start(out=out_d[:], in_=accB0[0:1, :])
                else:
                    nc.sync.dma_start(out=out_d[:], in_=accB0[0:N_SPANS, :])
    nc.compile()
    return nc


def _get_v2(sched, repeat=1):
    key = ("v3",) + _sched_key(sched, repeat)
    if key not in _cache:
        _cache[key] = _build_v2(sched, repeat)
    return _cache[key]


# ------------------------------------------------------------------- entry

def _to_bf16_padded(context):
    import ml_dtypes
    ctx16 = np.empty((B, S_PAD, H), dtype=ml_dtypes.bfloat16)
    ctx16[:, :S] = np.asarray(context, dtype=ml_dtypes.bfloat16)
    ctx16[:, S:] = ml_dtypes.bfloat16(-3.0e38)
    return ctx16


def kernel(context, spans_begin, spans_len):
    from concourse.bass_utils import run_bass_kernel_spmd

    spans_begin = np.asarray(spans_begin, dtype=np.int32)
    spans_len = np.asarray(spans_len, dtype=np.int32)
    assert spans_begin.shape == (B, N_SPANS)

    ctx16 = _to_bf16_padded(context)
    sched, metas, orders = _plan(spans_begin, spans_len)
    nc = _get_v2(sched)
    in_maps = [{"ctx": ctx16[b], "meta": metas[b]} for b in range(B)]
    res = run_bass_kernel_spmd(nc, in_maps, list(range(N_CORES)))
    out = np.empty((B, N_SPANS, H), dtype=np.float32)
    for b in range(B):
        out[b, orders[b]] = res.results[b]["out"].astype(np.float32)
    return out
